# revision 17
# baseline (speedup 1.0000x reference)
"""Haar DWT (2x2 stride-2 block decomposition) on 8 Trainium2 NeuronCores.

Input x: (32, 3, 512, 512) f32. Outputs (ll, lh, hl, hh): each (32, 3, 256, 256).

Sharding: pure data parallel over the batch dim — 4 images per core, viewed as
12 channel images of 512x512 per core, one channel per iteration.

Dataflow (v2): BOTH butterfly stages run on the TensorEngine via PSUM
accumulation, so no DVE tensor-tensor work remains:

  P1 =  W @ x_even + W @ x_odd   ->  [ll (p<64) ; lh (p>=64)]
  P2 = -W @ x_even + W @ x_odd   ->  [hl (p<64) ; hh (p>=64)]

where W is the 128x128 vertical row-pair butterfly (+-0.5 entries, exact) and
the even/odd column split comes from stride-2 moving-operand access patterns.
Matmuls run in float32r (1 col/cycle vs 4 for fp32) over two 128-row tiles at
a time (N=512 moving columns, the fp32 limit). Each PSUM bank is drained by a
single copy (alternating ACT/DVE) that also converts to fp16, halving the
store traffic. Loads are issued on the Sync HWDGE ring and stores on the ACT
HWDGE ring, leaving GpSimd idle.
"""

import sys

import numpy as np

if "/opt/trn_rl_repo" not in sys.path:
    sys.path.insert(0, "/opt/trn_rl_repo")

from concourse import bacc, bass, mybir
from concourse import tile
from concourse.bass_utils import run_bass_kernel_spmd

N_CORES = 8
B, C, H, W = 32, 3, 512, 512
BPC = B // N_CORES  # images per core
NCH = BPC * C  # channel images per core (12)
P = 128  # SBUF partitions
NT = H // P  # 128-row tiles per channel (4)
HW_OUT = H // 2  # 256

_CACHE = {}


def _butterfly_weights():
    """w[0] = vertical butterfly W (sum rows to p<64, diff rows to p>=64);
    w[1] = -W (for the hl/hh accumulation group's even-column pass)."""
    w = np.zeros((2, P, P), dtype=np.float32)
    for m in range(64):
        w[0, 2 * m, m] = 0.5
        w[0, 2 * m + 1, m] = 0.5
        w[0, 2 * m, 64 + m] = -0.5
        w[0, 2 * m + 1, 64 + m] = 0.5
    w[1] = -w[0]
    return w


def _build():
    nc = bacc.Bacc("TRN2", target_bir_lowering=False, debug=False)
    f32 = mybir.dt.float32
    f16 = mybir.dt.float16
    # x viewed as [NCH, tile, row-in-tile, W]
    x = nc.dram_tensor("x", [NCH, NT, P, W], f32, kind="ExternalInput")
    # +-0.5 is exact in fp16; fp16 matmuls stream 1 col/cycle (fp32 is 4x
    # slower) and enable the 4x fast weight load
    w = nc.dram_tensor("w", [2, P, P], f16, kind="ExternalInput")
    # out[ch, p, g, t, j]: g=0: [ll (p<64) | lh (p>=64)], g=1: [hl | hh];
    # output image row r = 64*t + (p mod 64)
    out = nc.dram_tensor("out", [NCH, P, 2, NT, HW_OUT], f16, kind="ExternalOutput")
    xa = x.ap()
    oa = out.ap()
    with tile.TileContext(nc) as tc:
        with (
            tc.tile_pool(name="p", bufs=5) as pool,
            tc.tile_pool(name="pc", bufs=5) as cpool,
            tc.tile_pool(name="o", bufs=3) as opool,
            tc.tile_pool(name="w", bufs=1) as wpool,
            tc.tile_pool(name="ps", bufs=4, space=bass.MemorySpace.PSUM) as psum,
        ):
            wt = wpool.tile([P, 2, P], f16)
            for i in range(NCH):
                # alternate loads across both HWDGE rings (sync + scalar) so
                # the load-only ramp phase gets both descriptor streams
                ldq = nc.sync if i % 2 == 0 else nc.scalar
                xin = pool.tile([P, NT, W], f32)
                xbf = cpool.tile([P, NT, W], f16)
                if i == 0:
                    # split the first load/cast so matmuls start early
                    for t in range(NT):
                        (nc.sync if t % 2 == 0 else nc.scalar).dma_start(
                            out=xin[:, t, :], in_=xa[i, t]
                        )
                    # weight load off the critical path of the first rows
                    nc.sync.dma_start(out=wt[:], in_=w.ap().transpose([1, 0, 2]))
                    for h in range(2):
                        nc.vector.tensor_copy(
                            xbf[:, 2 * h : 2 * h + 2, :], xin[:, 2 * h : 2 * h + 2, :]
                        )
                else:
                    # (t, p, w) -> (p, t, w); fully sequential DRAM read
                    ldq.dma_start(out=xin[:], in_=xa[i].transpose([1, 0, 2]))
                    # fp32 -> fp16 cast on DVE (2-port mode)
                    nc.vector.tensor_copy(xbf[:], xin[:])
                outt = opool.tile([P, 2, NT, HW_OUT], f16)
                for g in range(NT // 2):
                    t0 = 2 * g
                    # [p, tile-pair, parity, j]
                    ev = xbf[:, t0 : t0 + 2, :].rearrange(
                        "p t (j two) -> p t two j", two=2
                    )
                    # one 2-bank PSUM tile per group: [0:512]=ll|lh, [512:1024]=hl|hh
                    pt = psum.tile([P, 4 * HW_OUT], f32)
                    nc.tensor.matmul(
                        pt[:, 0:512], wt[:, 0, :], ev[:, :, 0, :], start=True, stop=False
                    )
                    nc.tensor.matmul(
                        pt[:, 0:512], wt[:, 0, :], ev[:, :, 1, :], start=False, stop=True
                    )
                    nc.tensor.matmul(
                        pt[:, 512:1024], wt[:, 1, :], ev[:, :, 0, :], start=True, stop=False
                    )
                    nc.tensor.matmul(
                        pt[:, 512:1024], wt[:, 0, :], ev[:, :, 1, :], start=False, stop=True
                    )
                    # drain both banks with one fp32->fp16 converting copy,
                    # alternating engines per group
                    src = pt[:].rearrange("p (b t j) -> p b t j", b=2, j=HW_OUT)
                    dst = outt[:, :, t0 : t0 + 2, :]
                    if g == 0:
                        nc.scalar.copy(dst, src)
                    else:
                        nc.vector.tensor_copy(dst, src)
                    if i >= NCH - 2:
                        # fine-grained tail: store each group as soon as its
                        # drain lands so the pipeline flushes quickly
                        nc.scalar.dma_start(
                            out=oa[i, :, :, t0 : t0 + 2, :],
                            in_=outt[:, :, t0 : t0 + 2, :],
                        )
                if i < NCH - 2:
                    nc.scalar.dma_start(out=oa[i], in_=outt[:])
    nc.compile()
    return nc


def _get_nc():
    if "nc" not in _CACHE:
        _CACHE["nc"] = _build()
    return _CACHE["nc"]


def run(x, **spmd_kwargs):
    """Run the DWT on 8 cores; returns (results_tuple, BassKernelResults)."""
    nc = _get_nc()
    xs = np.ascontiguousarray(np.asarray(x, dtype=np.float32)).reshape(
        N_CORES, NCH, NT, P, W
    )
    wmat = _butterfly_weights().astype(np.float16)
    in_maps = [{"x": xs[i], "w": wmat} for i in range(N_CORES)]
    res = None
    for attempt in range(3):
        try:
            res = run_bass_kernel_spmd(
                nc, in_maps, core_ids=list(range(N_CORES)), **spmd_kwargs
            )
            break
        except Exception:
            # transient device wedge (NRT_EXEC_UNIT_UNRECOVERABLE) recovers
            # on retry; re-raise only if it persists
            if attempt == 2:
                raise
            import time

            time.sleep(2)
    # per-core out: (NCH, P, 2, NT, HW_OUT) fp16
    full = np.stack([res.results[i]["out"] for i in range(N_CORES)])

    def expand(g, half):  # -> (B, C, 256, 256) f32
        sl = full[:, :, 64 * half : 64 * (half + 1), g]  # (cores, NCH, 64, NT, j)
        sl = sl.transpose(0, 1, 3, 2, 4)  # row r = 64*t + p64
        return np.ascontiguousarray(sl, dtype=np.float32).reshape(B, C, HW_OUT, HW_OUT)

    ll = expand(0, 0)
    lh = expand(0, 1)
    hl = expand(1, 0)
    hh = expand(1, 1)
    return (ll, lh, hl, hh), res


def kernel(x):
    out, _ = run(x)
    return out


# revision 18
# speedup vs baseline: 1.2018x; 1.2018x over previous
"""Haar DWT (2x2 stride-2 block decomposition) on 8 Trainium2 NeuronCores.

Input x: (32, 3, 512, 512) f32. Outputs (ll, lh, hl, hh): each (32, 3, 256, 256).

Sharding: pure data parallel over the batch dim — 4 images per core, viewed as
12 channel images of 512x512 per core, one channel per iteration.

Dataflow (v2): BOTH butterfly stages run on the TensorEngine via PSUM
accumulation, so no DVE tensor-tensor work remains:

  P1 =  W @ x_even + W @ x_odd   ->  [ll (p<64) ; lh (p>=64)]
  P2 = -W @ x_even + W @ x_odd   ->  [hl (p<64) ; hh (p>=64)]

where W is the 128x128 vertical row-pair butterfly (+-0.5 entries, exact) and
the even/odd column split comes from stride-2 moving-operand access patterns.
Matmuls run in float32r (1 col/cycle vs 4 for fp32) over two 128-row tiles at
a time (N=512 moving columns, the fp32 limit). Each PSUM bank is drained by a
single copy (alternating ACT/DVE) that also converts to fp16, halving the
store traffic. Loads are issued on the Sync HWDGE ring and stores on the ACT
HWDGE ring, leaving GpSimd idle.
"""

import sys

import numpy as np

if "/opt/trn_rl_repo" not in sys.path:
    sys.path.insert(0, "/opt/trn_rl_repo")

from concourse import bacc, bass, mybir
from concourse import tile
from concourse.bass_utils import run_bass_kernel_spmd

N_CORES = 8
B, C, H, W = 32, 3, 512, 512
BPC = B // N_CORES  # images per core
NCH = BPC * C  # channel images per core (12)
P = 128  # SBUF partitions
NT = H // P  # 128-row tiles per channel (4)
HW_OUT = H // 2  # 256

_CACHE = {}


def _butterfly_weights():
    """w[0] = vertical butterfly W (sum rows to p<64, diff rows to p>=64);
    w[1] = -W (for the hl/hh accumulation group's even-column pass)."""
    w = np.zeros((2, P, P), dtype=np.float32)
    for m in range(64):
        w[0, 2 * m, m] = 0.5
        w[0, 2 * m + 1, m] = 0.5
        w[0, 2 * m, 64 + m] = -0.5
        w[0, 2 * m + 1, 64 + m] = 0.5
    w[1] = -w[0]
    return w


def _build():
    nc = bacc.Bacc("TRN2", target_bir_lowering=False, debug=False)
    f32 = mybir.dt.float32
    f16 = mybir.dt.float16
    # x viewed as [NCH, tile, row-in-tile, W]
    x = nc.dram_tensor("x", [NCH, NT, P, W], f32, kind="ExternalInput")
    # +-0.5 is exact in fp16; fp16 matmuls stream 1 col/cycle (fp32 is 4x
    # slower) and enable the 4x fast weight load
    w = nc.dram_tensor("w", [2, P, P], f16, kind="ExternalInput")
    # out[ch, p, g, t, j]: g=0: [ll (p<64) | lh (p>=64)], g=1: [hl | hh];
    # output image row r = 64*t + (p mod 64)
    out = nc.dram_tensor("out", [NCH, P, 2, NT, HW_OUT], f16, kind="ExternalOutput")
    xa = x.ap()
    oa = out.ap()
    with tile.TileContext(nc) as tc:
        with (
            tc.tile_pool(name="p", bufs=5) as pool,
            tc.tile_pool(name="pc", bufs=5) as cpool,
            tc.tile_pool(name="o", bufs=3) as opool,
            tc.tile_pool(name="w", bufs=1) as wpool,
            tc.tile_pool(name="ps", bufs=4, space=bass.MemorySpace.PSUM) as psum,
        ):
            wt = wpool.tile([P, 2, P], f16)
            for i in range(NCH):
                xin = pool.tile([P, NT, W], f32)
                xbf = cpool.tile([P, NT, W], f16)
                if i == 0:
                    # split the first load/cast so matmuls start early
                    for t in range(NT):
                        nc.sync.dma_start(out=xin[:, t, :], in_=xa[i, t])
                    # weight load off the critical path of the first rows
                    nc.sync.dma_start(out=wt[:], in_=w.ap().transpose([1, 0, 2]))
                    for h in range(2):
                        nc.vector.tensor_copy(
                            xbf[:, 2 * h : 2 * h + 2, :], xin[:, 2 * h : 2 * h + 2, :]
                        )
                else:
                    # (t, p, w) -> (p, t, w); fully sequential DRAM read
                    nc.sync.dma_start(out=xin[:], in_=xa[i].transpose([1, 0, 2]))
                    # fp32 -> fp16 cast on DVE (2-port mode)
                    nc.vector.tensor_copy(xbf[:], xin[:])
                outt = opool.tile([P, 2, NT, HW_OUT], f16)
                for g in range(NT // 2):
                    t0 = 2 * g
                    # [p, tile-pair, parity, j]
                    ev = xbf[:, t0 : t0 + 2, :].rearrange(
                        "p t (j two) -> p t two j", two=2
                    )
                    # one 2-bank PSUM tile per group: [0:512]=ll|lh, [512:1024]=hl|hh
                    pt = psum.tile([P, 4 * HW_OUT], f32)
                    nc.tensor.matmul(
                        pt[:, 0:512], wt[:, 0, :], ev[:, :, 0, :], start=True, stop=False
                    )
                    nc.tensor.matmul(
                        pt[:, 0:512], wt[:, 0, :], ev[:, :, 1, :], start=False, stop=True
                    )
                    nc.tensor.matmul(
                        pt[:, 512:1024], wt[:, 1, :], ev[:, :, 0, :], start=True, stop=False
                    )
                    nc.tensor.matmul(
                        pt[:, 512:1024], wt[:, 0, :], ev[:, :, 1, :], start=False, stop=True
                    )
                    # drain both banks with one fp32->fp16 converting copy,
                    # alternating engines per group
                    src = pt[:].rearrange("p (b t j) -> p b t j", b=2, j=HW_OUT)
                    dst = outt[:, :, t0 : t0 + 2, :]
                    if g == 0:
                        nc.scalar.copy(dst, src)
                    else:
                        nc.vector.tensor_copy(dst, src)
                    if i >= NCH - 2:
                        # fine-grained tail: store each group as soon as its
                        # drain lands so the pipeline flushes quickly
                        nc.scalar.dma_start(
                            out=oa[i, :, :, t0 : t0 + 2, :],
                            in_=outt[:, :, t0 : t0 + 2, :],
                        )
                if i < NCH - 2:
                    nc.scalar.dma_start(out=oa[i], in_=outt[:])
    nc.compile()
    return nc


def _get_nc():
    if "nc" not in _CACHE:
        _CACHE["nc"] = _build()
    return _CACHE["nc"]


def run(x, **spmd_kwargs):
    """Run the DWT on 8 cores; returns (results_tuple, BassKernelResults)."""
    nc = _get_nc()
    xs = np.ascontiguousarray(np.asarray(x, dtype=np.float32)).reshape(
        N_CORES, NCH, NT, P, W
    )
    wmat = _butterfly_weights().astype(np.float16)
    in_maps = [{"x": xs[i], "w": wmat} for i in range(N_CORES)]
    res = None
    for attempt in range(3):
        try:
            res = run_bass_kernel_spmd(
                nc, in_maps, core_ids=list(range(N_CORES)), **spmd_kwargs
            )
            break
        except Exception:
            # transient device wedge (NRT_EXEC_UNIT_UNRECOVERABLE) recovers
            # on retry; re-raise only if it persists
            if attempt == 2:
                raise
            import time

            time.sleep(2)
    # per-core out: (NCH, P, 2, NT, HW_OUT) fp16
    full = np.stack([res.results[i]["out"] for i in range(N_CORES)])

    def expand(g, half):  # -> (B, C, 256, 256) f32
        sl = full[:, :, 64 * half : 64 * (half + 1), g]  # (cores, NCH, 64, NT, j)
        sl = sl.transpose(0, 1, 3, 2, 4)  # row r = 64*t + p64
        return np.ascontiguousarray(sl, dtype=np.float32).reshape(B, C, HW_OUT, HW_OUT)

    ll = expand(0, 0)
    lh = expand(0, 1)
    hl = expand(1, 0)
    hh = expand(1, 1)
    return (ll, lh, hl, hh), res


def kernel(x):
    out, _ = run(x)
    return out
